# revision 2
# baseline (speedup 1.0000x reference)
"""GCN layer (fc + gather/scatter-sum) on 8 trn2 NeuronCores — v3.

Same architecture as the baseline kernel.py (aggregate raw bf16 features
over edges with the SWDGE dma_gather, then fold the FC after aggregation),
but with PACKED slot layouts: per-(block, group) segments are budgeted at
the exact max-over-cores edge count (no 128-slot tile alignment), packed
back-to-back within each (superbatch, group) gather call.  Slot windows of
128 that straddle a segment boundary get one scatter matmul per overlapped
block (the A one-hot strip carries one column block per (window, block)
overlap), instead of padding every segment to a tile multiple.  This cuts
~11% of the gather descriptors, which is the dominant cost: SWDGE
descriptor generation on the Q7 cluster (~4ns/desc, cluster-serialized) is
the kernel's critical path.
"""

import os
import numpy as np
import ml_dtypes

import concourse.bass as bass
import concourse.bacc as bacc
import concourse.mybir as mybir
from concourse import tile

P = 128
NCORES = 8


def _patch_tile_exit():
    """The walrus build in this container rejects two constructs Tile emits
    at TileContext exit (see baseline kernel.py)."""
    import bass_rust
    from concourse.vector_clock import ScopedClock

    def _drain_and_barrier(self, tick_clock, wait_clock):
        drain_inst = self.nc.sync.drain()
        wait_clock.add_sem_waits(
            drain_inst.ins, ScopedClock({None: tick_clock.global_clock})
        )
        si = drain_inst.ins.sync_info
        if si is not None and len(si.on_wait) > 1:
            waits = list(si.on_wait)
            drain_inst.ins.sync_info = bass_rust.SyncInfo(
                on_wait=waits[:1], on_update=list(si.on_update))
            for w in waits[1:]:
                extra = self.nc.sync.drain()
                extra.ins.sync_info = bass_rust.SyncInfo(
                    on_wait=[w], on_update=[])
        self.nc.all_engine_barrier()
        popped = self.nc._tile_sem_poison_stack.pop()
        assert popped is self._sem_poison
        self.nc.all_engine_barrier()

    tile.TileContext._drain_and_barrier = _drain_and_barrier


_patch_tile_exit()


class Cfg:
    def __init__(self, n_nodes, d_in, d_out, ncores, group_shift, sb_blocks):
        self.N = n_nodes
        self.D = d_in
        self.DO = d_out
        self.ncores = ncores
        self.gshift = group_shift
        self.gsize = 1 << group_shift
        self.ngroups = (n_nodes + self.gsize - 1) >> group_shift
        self.npc = n_nodes // ncores
        self.nblk = (self.npc + P - 1) // P
        self.sb = sb_blocks


FULL_CFG = Cfg(n_nodes=100000, d_in=256, d_out=64, ncores=8, group_shift=15,
               sb_blocks=4)


def _round16(x):
    # call sizes stay 128-aligned: the SWDGE gather ucode's partial-chunk
    # dummy-descriptor path is unproven on this HW (the tile-aligned baseline
    # never exercised it), and a partial tail wedged the device in testing.
    return (x + 127) & ~127


def _prep_host(feature, W, b, src, dst, cfg):
    """Shard + sort edges; build packed slot layout and boundary-window
    matmul schedule (shared across cores via max-count budgets)."""
    N, npc, nblk, ng = cfg.N, cfg.npc, cfg.nblk, cfg.ngroups
    src = np.asarray(src, dtype=np.int64)
    dst = np.asarray(dst, dtype=np.int64)

    per_core = []
    counts = np.zeros((cfg.ncores, nblk, ng), dtype=np.int64)
    for m in range(cfg.ncores):
        lo, hi = m * npc, (m + 1) * npc
        mask = (dst >= lo) & (dst < hi)
        es = src[mask]
        ed = dst[mask] - lo
        blk = ed >> 7
        grp = es >> cfg.gshift
        order = np.lexsort((es, grp, blk))
        es, ed, blk, grp = es[order], ed[order], blk[order], grp[order]
        np.add.at(counts[m], (blk, grp), 1)
        per_core.append((es, ed, blk, grp))

    # shared budgets: exact max-over-cores count per (block, group)
    seg = counts.max(axis=0)  # [nblk, ng]
    for k in range(nblk):
        if seg[k].sum() == 0:
            seg[k][0] = 1

    # layout: superbatch -> group -> packed segments; call sizes 16-aligned
    nsb = (nblk + cfg.sb - 1) // cfg.sb
    call_offsets = {}     # (s, g) -> (slot_start, call_len)
    seg_start = {}        # (k, g) -> absolute slot start of segment
    # per (s, g): list of (window_idx_in_call, acol, k) matmul entries
    call_mm = {}
    # per (s, g): number of A columns (strip width in windows)
    call_acols = {}
    pos = 0
    for s in range(nsb):
        ks = list(range(s * cfg.sb, min((s + 1) * cfg.sb, nblk)))
        for g in range(ng):
            call0 = pos
            segs = []          # (k, lo, hi) call-relative
            cur = 0
            for k in ks:
                c = int(seg[k][g])
                if c == 0:
                    continue
                seg_start[(k, g)] = call0 + cur
                segs.append((k, cur, cur + c))
                cur += c
            size = _round16(cur)
            call_offsets[(s, g)] = (call0, size)
            # windows & overlaps
            mm = []
            acol = 0
            nwin = (size + P - 1) // P
            si = 0
            for w in range(nwin):
                wlo, whi = w * P, min((w + 1) * P, size)
                while si < len(segs) and segs[si][2] <= wlo:
                    si += 1
                j = si
                while j < len(segs) and segs[j][1] < whi:
                    mm.append((w, acol, segs[j][0]))
                    acol += 1
                    j += 1
            call_mm[(s, g)] = mm
            call_acols[(s, g)] = max(acol, 1)
            pos += size
    tot = pos
    assert tot % 16 == 0

    # A-strip column layout: strips are per (s, g), placed back-to-back in
    # one dstrel table of width sum(call_acols)
    acol_base = {}
    apos = 0
    for s in range(nsb):
        for g in range(ng):
            acol_base[(s, g)] = apos
            apos += call_acols[(s, g)]
    tot_acols = apos

    # matmul totals per block (for start/stop flags)
    win_per_block = np.zeros(nblk, dtype=np.int64)
    for (s, g), mm in call_mm.items():
        for (w, ac, k) in mm:
            win_per_block[k] += 1

    in_maps = []
    ftab = np.ascontiguousarray(feature.astype(ml_dtypes.bfloat16))
    wmat = np.ascontiguousarray(W.astype(np.float32))
    bbc = np.ascontiguousarray(np.tile(b.astype(np.float32)[None, :], (P, 1)))
    iota = np.ascontiguousarray(
        np.tile(np.arange(P, dtype=np.float32)[None, :], (P, 1)).astype(
            ml_dtypes.bfloat16))
    ident = np.eye(P, dtype=np.float32)
    ones = np.ones((P, 1), dtype=ml_dtypes.bfloat16)

    for m in range(cfg.ncores):
        es, ed, blk, grp = per_core[m]
        idx_arr = np.zeros(tot, dtype=np.int16)
        dst_arr = np.full((P, tot_acols), -1.0, dtype=np.float32)
        bounds = np.searchsorted(blk * ng + grp, np.arange(nblk * ng + 1))
        for s in range(nsb):
            for g in range(ng):
                for k in range(s * cfg.sb, min((s + 1) * cfg.sb, nblk)):
                    a, bnd = bounds[k * ng + g], bounds[k * ng + g + 1]
                    n = bnd - a
                    if n == 0:
                        continue
                    s0 = seg_start[(k, g)]
                    idx_arr[s0:s0 + n] = (es[a:bnd] - (g << cfg.gshift)
                                          ).astype(np.int16)
                # dstrel per (window, k) A-column
                call0, size = call_offsets[(s, g)]
                ab = acol_base[(s, g)]
                for (w, ac, k) in call_mm[(s, g)]:
                    a, bnd = bounds[k * ng + g], bounds[k * ng + g + 1]
                    n = bnd - a
                    s0 = seg_start[(k, g)]
                    # slots of this (k,g) segment that fall in window w
                    wlo = call0 + w * P
                    whi = wlo + P
                    elo = max(s0, wlo)
                    ehi = min(s0 + n, whi)   # only real edges (not holes)
                    if ehi > elo:
                        ei = elo - s0        # edge index within segment
                        rr = np.arange(elo - wlo, ehi - wlo)
                        dst_arr[rr, ab + ac] = (
                            ed[a + ei:a + ei + (ehi - elo)] - k * P
                        ).astype(np.float32)
        idx16 = np.ascontiguousarray(
            np.tile(idx_arr.reshape(tot // 16, 16).T, (P // 16, 1)))
        dstrel = np.ascontiguousarray(dst_arr.astype(ml_dtypes.bfloat16))
        deg = np.zeros(nblk * P, dtype=np.float32)
        np.add.at(deg, ed, 1.0)
        degc = np.ascontiguousarray(deg.reshape(nblk, P).T)
        in_maps.append({
            "ftab": ftab, "idx16": idx16, "dstrel": dstrel,
            "wmat": wmat, "bbc": bbc, "iota": iota, "ident": ident,
            "ones": ones, "degc": degc,
        })

    meta = dict(call_offsets=call_offsets, call_mm=call_mm,
                call_acols=call_acols, acol_base=acol_base,
                win_per_block=win_per_block, tot=tot, tot_acols=tot_acols,
                nsb=nsb)
    return in_maps, meta


def _build_program(cfg, meta):
    N, D, DO, nblk, ng = cfg.N, cfg.D, cfg.DO, cfg.nblk, cfg.ngroups
    call_offsets, call_mm = meta["call_offsets"], meta["call_mm"]
    call_acols, acol_base = meta["call_acols"], meta["acol_base"]
    win_per_block = meta["win_per_block"]
    tot, tot_acols, nsb = meta["tot"], meta["tot_acols"], meta["nsb"]
    bf16, f32, i16 = mybir.dt.bfloat16, mybir.dt.float32, mybir.dt.int16

    nc = bacc.Bacc(None, target_bir_lowering=False, num_swdge_queues=4)
    ftab = nc.dram_tensor("ftab", [N, D], bf16, kind="ExternalInput")
    idx16 = nc.dram_tensor("idx16", [P, tot // 16], i16, kind="ExternalInput")
    dstrel = nc.dram_tensor("dstrel", [P, tot_acols], bf16,
                            kind="ExternalInput")
    wmat = nc.dram_tensor("wmat", [D, DO], f32, kind="ExternalInput")
    bbc = nc.dram_tensor("bbc", [P, DO], f32, kind="ExternalInput")
    iota = nc.dram_tensor("iota", [P, P], bf16, kind="ExternalInput")
    ident = nc.dram_tensor("ident", [P, P], f32, kind="ExternalInput")
    ones = nc.dram_tensor("ones", [P, 1], bf16, kind="ExternalInput")
    degc = nc.dram_tensor("degc", [P, nblk], f32, kind="ExternalInput")
    out = nc.dram_tensor("out", [cfg.npc, DO], f32, kind="ExternalOutput")

    kchunks = D // P

    with tile.TileContext(nc) as tc:
        with (
            tc.tile_pool(name="const", bufs=1) as cpool,
            tc.tile_pool(name="gath", bufs=2) as gpool,
            tc.tile_pool(name="amat", bufs=2) as apool,
            tc.tile_pool(name="work", bufs=4) as wpool,
            tc.tile_pool(name="psag", bufs=cfg.sb, space="PSUM") as psag,
            tc.tile_pool(name="pstr", bufs=2, space="PSUM") as pstr,
            tc.tile_pool(name="psout", bufs=2, space="PSUM") as psout,
        ):
            gsz_regs = [nc.alloc_register(mybir.EngineType.Pool, f"gsz{q}")
                        for q in range(4)]
            call_no = 0
            idxt = cpool.tile([P, tot // 16], i16)
            nc.sync.dma_start(out=idxt[:], in_=idx16[:])
            dstt = cpool.tile([P, tot_acols], bf16)
            nc.sync.dma_start(out=dstt[:], in_=dstrel[:])
            iotat = cpool.tile([P, P], bf16)
            nc.sync.dma_start(out=iotat[:], in_=iota[:])
            identt = cpool.tile([P, P], f32)
            nc.sync.dma_start(out=identt[:], in_=ident[:])
            onest = cpool.tile([P, 1], bf16)
            nc.sync.dma_start(out=onest[:], in_=ones[:])
            degt = cpool.tile([P, nblk], f32)
            nc.sync.dma_start(out=degt[:], in_=degc[:])
            bbct = cpool.tile([P, DO], f32)
            nc.sync.dma_start(out=bbct[:], in_=bbc[:])
            wts = []
            for c in range(kchunks):
                wt = cpool.tile([P, DO], f32, tag=f"w{c}")
                nc.sync.dma_start(out=wt[:], in_=wmat[c * P:(c + 1) * P, :])
                wts.append(wt)

            mm_count = np.zeros(nblk, dtype=np.int64)
            agg = {}
            for s in range(nsb):
                # gather calls + A strips for this superbatch
                gts = {}
                for g in range(ng):
                    off, size = call_offsets[(s, g)]
                    if size == 0:
                        continue
                    ntile = (size + P - 1) // P
                    gt = gpool.tile([P, ntile * D], bf16, tag=f"g{g}",
                                    name=f"gt{s}_{g}")
                    gt3 = gt[:].rearrange("p (t e) -> p t e", e=D)
                    glo = g << cfg.gshift
                    ghi = min(glo + cfg.gsize, N)
                    if os.environ.get("GCN_SKIP_GATHER"):
                        nc.vector.memset(gt[:, 0:1], 0.0)
                    else:
                        q = call_no % 4
                        call_no += 1
                        nc.gpsimd.reg_mov(gsz_regs[q], size)
                        nc.gpsimd.dma_gather(
                            out_ap=gt3,
                            in_ap=ftab[glo:ghi, :],
                            idxs_ap=idxt[:, off // 16:(off + size) // 16],
                            num_idxs=size,
                            num_idxs_reg=gsz_regs[q],
                            elem_size=D,
                            single_packet=(size <= 1024),
                            queue_num=q,
                        )
                    gts[g] = gt3

                    # A strip for this call: one is_equal over its columns
                    nac = call_acols[(s, g)]
                    ab = acol_base[(s, g)]
                    at = apool.tile([P, nac * P], bf16, tag=f"a{g}",
                                    name=f"at{s}_{g}")
                    d_b = dstt[:, ab:ab + nac].to_broadcast([P, nac, P])
                    iap = iotat[:]
                    i_b = bass.AP(iap.tensor, iap.offset,
                                  [iap.ap[0], [0, nac], iap.ap[1]])
                    nc.vector.tensor_tensor(
                        out=at[:].rearrange("p (t d) -> p t d", d=P),
                        in0=i_b, in1=d_b, op=mybir.AluOpType.is_equal)
                    gts[g] = (gt3, at)

                # scatter matmuls in (g, window) order
                for g in range(ng):
                    if (s, g) not in call_mm:
                        continue
                    if call_offsets[(s, g)][1] == 0:
                        continue
                    gt3, at = gts[g]
                    for (w, ac, k) in call_mm[(s, g)]:
                        if k not in agg:
                            agg[k] = psag.tile([P, D], f32, tag="agg",
                                               name=f"agg{k}")
                        first = mm_count[k] == 0
                        last = mm_count[k] == win_per_block[k] - 1
                        nc.tensor.matmul(
                            agg[k][:, 0:D],
                            lhsT=at[:, ac * P:(ac + 1) * P],
                            rhs=gt3[:, w, :],
                            start=bool(first), stop=bool(last))
                        mm_count[k] += 1
                        if not last:
                            continue
                        # FC for this block: out_blk = agg @ W + deg * b
                        aggs = wpool.tile([P, D], f32, tag="aggs",
                                          name=f"aggs{k}")
                        nc.scalar.copy(out=aggs[:], in_=agg[k][:, 0:D])
                        del agg[k]
                        po = psout.tile([P, DO], f32, tag="po",
                                        name=f"po{k}")
                        for c in range(kchunks):
                            pt = pstr.tile([P, P], f32, tag="pt",
                                           name=f"pt{k}_{c}")
                            nc.tensor.transpose(
                                pt[:], aggs[:, c * P:(c + 1) * P], identt[:])
                            aT = wpool.tile([P, P], f32, tag="aT",
                                            name=f"aT{k}_{c}")
                            nc.vector.tensor_copy(out=aT[:], in_=pt[:])
                            nc.tensor.matmul(po[:], lhsT=aT[:],
                                             rhs=wts[c][:], start=(c == 0),
                                             stop=(c == kchunks - 1))
                        bias = wpool.tile([P, DO], f32, tag="bias",
                                          name=f"bias{k}")
                        nc.vector.tensor_tensor(
                            out=bias[:], in0=bbct[:],
                            in1=degt[:, k:k + 1].to_broadcast([P, DO]),
                            op=mybir.AluOpType.mult)
                        outt = wpool.tile([P, DO], f32, tag="outt",
                                          name=f"outt{k}")
                        nc.vector.tensor_tensor(
                            out=outt[:], in0=po[:], in1=bias[:],
                            op=mybir.AluOpType.add)
                        rows = min(P, cfg.npc - k * P)
                        nc.sync.dma_start(out=out[k * P:k * P + rows, :],
                                          in_=outt[:rows, :])
            assert (mm_count == win_per_block).all()
    return nc


def _run_spmd(nc, in_maps, trace=False):
    from concourse.bass_utils import run_bass_kernel_spmd
    return run_bass_kernel_spmd(nc, in_maps, list(range(len(in_maps))),
                                trace=trace)


_PROGRAM_CACHE = {}


def gcn_kernel(feature, W, b, src, dst, cfg=FULL_CFG, trace=False):
    in_maps, meta = _prep_host(feature, W, b, src, dst, cfg)
    key = (cfg.N, meta["tot"], meta["tot_acols"],
           tuple(sorted((k, v[0], v[1])
                        for k, v in meta["call_offsets"].items())),
           tuple(sorted((k, tuple(v)) for k, v in meta["call_mm"].items())))
    nc = _PROGRAM_CACHE.get(key)
    if nc is None:
        nc = _build_program(cfg, meta)
        nc.finalize()
        _PROGRAM_CACHE[key] = nc
    res = _run_spmd(nc, in_maps, trace=trace)
    outs = [res.results[m]["out"] for m in range(cfg.ncores)]
    full = np.concatenate(outs, axis=0).astype(np.float32)
    return full, res


def kernel(**inputs):
    feature = np.asarray(inputs["feature"], dtype=np.float32)
    W = np.asarray(inputs["W"], dtype=np.float32)
    b = np.asarray(inputs["b"], dtype=np.float32)
    src = np.asarray(inputs["src"], dtype=np.int32)
    dst = np.asarray(inputs["dst"], dtype=np.int32)
    full, _ = gcn_kernel(feature, W, b, src, dst, FULL_CFG)
    return full
